# revision 17
# baseline (speedup 1.0000x reference)
"""DownBlock kernel for trn2 (8-core SPMD, Bass/Tile), v2.

Vertex-sharded: each core owns Vs=5248 coarse vertices x all 16 batches.
Pipeline per core:
  P1 pool:  dedup'd int16 dma_gather (bf16 1024B rows) + k-reduce -> xp_shard
  AG xp (bf16)
  C1 conv1: complementary A/B int16 gathers (zero-fill negatives) + DVE add,
            DVE transpose to (k,c)-partitions, bf16 matmuls with k folded
            into partitions (2 k-groups), Act dumps for BN stats, DVE "ht"
            transpose -> h_raw rows (v, (16b,64o)) bf16
  AR stats -> BN1 coeffs; P2 apply (scale+bias+lrelu) -> h_shard; AG h (bf16)
  C2 conv2: complementary TRANSPOSE-mode gathers (dummy zero-row for
            invalid), partitions become (2b,64o); block-diagonal bf16
            weights contract (k,o) with b_lo carried; psum (2b,64o2) x
            (8b_hi,64v); Act dumps (stats2 + staging); direct out writes
  AR stats2 -> BN2 coeffs; P4 fixup: Prelu(scale,bias) read-modify-write out.
"""
import sys

sys.path.insert(0, "/opt/trn_rl_repo")

import contextlib

import numpy as np
import ml_dtypes

import concourse.bass as bass
import concourse.bacc as bacc
import concourse.mybir as mybir
import concourse.tile as tile
from concourse.masks import make_identity

F32 = mybir.dt.float32
BF16 = mybir.dt.bfloat16
I16 = mybir.dt.int16
AF = mybir.ActivationFunctionType
OP = mybir.AluOpType
EPS = 1e-5
ALPHA = 0.2
P = 128


class Cfg:
    def __init__(self, B=16, C1=32, C2=64, K=7, VF=163842, VC=40962, n_cores=8,
                 stop_after=None):
        self.stop_after = stop_after
        self.B, self.C1, self.C2, self.K = B, C1, C2, K
        self.VF, self.VC, self.n_cores = VF, VC, n_cores
        self.R1 = B * C1          # 512
        self.R2 = B * C2          # 1024
        tiles_total = -(-VC // P)
        self.TPC = -(-tiles_total // n_cores)   # 41
        self.VC_pad = self.TPC * n_cores * P
        self.Vs = self.TPC * P                  # 5248
        self.H = (self.Vs + 1) * (n_cores // 2)  # split point 20996
        assert self.H < 32767 and (self.Vs + 1) * n_cores - self.H < 32767


def _pack16(vals, nidx):
    """Pack nidx int16 idx values: j -> partition j%16, col j//16, tiled x8."""
    blk = vals.reshape(nidx // 16, 16).T
    return np.tile(blk, (8, 1)).astype(np.int16)


def host_prep(cfg, x, w1, w2, g1, beta1, g2, beta2, conv_neigh, down_neigh):
    B, C1, C2, K = cfg.B, cfg.C1, cfg.C2, cfg.K
    VF, VC, Vs, TPC, H = cfg.VF, cfg.VC, cfg.Vs, cfg.TPC, cfg.H
    x = np.asarray(x, np.float32)
    conv_neigh = np.asarray(conv_neigh).astype(np.int64)
    down_neigh = np.asarray(down_neigh).astype(np.int64)
    w1 = np.asarray(w1, np.float32)
    w2 = np.asarray(w2, np.float32)

    xT = np.ascontiguousarray(
        x.transpose(2, 0, 1).reshape(VF, B * C1)).astype(ml_dtypes.bfloat16)

    # W1 k-grouped: kg0 rows (k<4)*32+c, kg1 rows (k-4)*32+c; value w1[o,c*K+k]/K
    w1r = w1.reshape(C2, C1, K) / K        # [o, c, k]
    W1kg0 = np.zeros((128, C2), np.float32)
    W1kg1 = np.zeros((128, C2), np.float32)
    for k in range(4):
        W1kg0[k * 32:(k + 1) * 32, :] = w1r[:, :, k].T
    for k in range(4, 7):
        W1kg1[(k - 4) * 32:(k - 3) * 32, :] = w1r[:, :, k].T
    # W2 per-k block-diagonal over b_lo(2): [(bl*64+o),(bl*64+o2)] = w2[o2,o*K+k]
    w2r = w2.reshape(C2, C2, K)            # [o2, o, k]
    W2k = np.zeros((K, 128, 128), np.float32)
    for k in range(K):
        for bl in range(2):
            W2k[k, bl * 64:(bl + 1) * 64, bl * 64:(bl + 1) * 64] = w2r[:, :, k].T

    gvec = np.stack([np.asarray(g1, np.float32), np.asarray(beta1, np.float32),
                     np.asarray(g2, np.float32), np.asarray(beta2, np.float32)],
                    axis=1)

    NIDX = K * P
    per_core = []
    for ci in range(cfg.n_cores):
        sh0 = ci * Vs
        vid = sh0 + np.arange(Vs)
        valid = vid < VC
        vv = np.where(valid, vid, 0)
        pool_idx = down_neigh[vv].copy()
        pool_idx[~valid] = 0
        ci_raw = conv_neigh[vv]
        gidx = (ci_raw // Vs) * (Vs + 1) + ci_raw % Vs   # into (Vs+1)*8 table
        # pad vertices -> core0 zero row (index Vs)
        gidx[~valid] = Vs

        # pool: dedup per half (int16 range)
        TH0 = (TPC + 1) // 2
        xt_halves, pool_packed = [], []
        for t0, t1 in ((0, TH0), (TH0, TPC)):
            sl = pool_idx[t0 * P:t1 * P, :]
            uniq, inv = np.unique(sl, return_inverse=True)
            assert len(uniq) < 32700
            xt_halves.append(xT[uniq])
            pidx = inv.reshape(sl.shape).astype(np.int16)
            for t in range(t1 - t0):
                lst = pidx[t * P:(t + 1) * P, :].T.reshape(NIDX)
                pool_packed.append(_pack16(lst, NIDX))
        # conv1: s-slot stream (1024 items): slot s=(khi*4+vhi), partition
        # p=(g*32+vlo), k=khi*4+g, v_local=vhi*32+vlo. k==7 -> -1 (zero-fill).
        s_g = np.arange(1024)
        s_s, s_p = s_g // 128, s_g % 128
        s_khi, s_vhi = s_s // 4, s_s % 4
        s_gg, s_vlo = s_p // 32, s_p % 32
        s_k = s_khi * 4 + s_gg
        s_vl = s_vhi * 32 + s_vlo
        s_valid = s_k < K
        # conv2: natural k-major stream (896 items): j = k*128 + vlo
        n_g = np.arange(NIDX)
        n_k, n_vl = n_g // P, n_g % P
        cA1, cB1, cA2, cB2 = [], [], [], []
        B_dummy = 5 * (Vs + 1) - 1 - H   # core-4 zero row, local to B table
        for t in range(TPC):
            blk = gidx[t * P:(t + 1) * P, :]          # (128 v, K)
            l1 = np.where(s_valid, blk[s_vl, np.minimum(s_k, K - 1)],
                          np.int64(Vs))
            a = np.where(l1 < H, l1, Vs)
            b_ = np.where(l1 >= H, l1 - H, B_dummy)
            cA1.append(_pack16(a.astype(np.int16), 1024))
            cB1.append(_pack16(b_.astype(np.int16), 1024))
            l2 = blk[n_vl, n_k]
            a2 = np.where(l2 < H, l2, Vs)
            b2 = np.where(l2 >= H, l2 - H, B_dummy)
            cA2.append(_pack16(a2.astype(np.int16), NIDX))
            cB2.append(_pack16(b2.astype(np.int16), NIDX))
        per_core.append(dict(
            xt0=xt_halves[0], xt1=xt_halves[1],
            pool_idx=np.concatenate(pool_packed, 0),
            convA1=np.concatenate(cA1, 0), convB1=np.concatenate(cB1, 0),
            convA2=np.concatenate(cA2, 0), convB2=np.concatenate(cB2, 0),
            w1kg0=W1kg0.astype(ml_dtypes.bfloat16),
            w1kg1=W1kg1.astype(ml_dtypes.bfloat16),
            w2k=W2k.reshape(K * 128, 128).astype(ml_dtypes.bfloat16),
            gvec=gvec,
        ))
    mx = max(max(pc["xt0"].shape[0], pc["xt1"].shape[0]) for pc in per_core)
    for pc in per_core:
        for nm in ("xt0", "xt1"):
            n = pc[nm].shape[0]
            if n < mx:
                pc[nm] = np.concatenate(
                    [pc[nm], np.zeros((mx - n, cfg.R1), ml_dtypes.bfloat16)], 0)
            pc[nm] = np.ascontiguousarray(pc[nm])
    return mx, per_core


def build(cfg, xt_rows_max):
    B, C2, K, R1, R2 = cfg.B, cfg.C2, cfg.K, cfg.R1, cfg.R2
    Vs, TPC, H, NC = cfg.Vs, cfg.TPC, cfg.H, cfg.n_cores
    NIDX = K * P
    nc = bacc.Bacc("TRN2", target_bir_lowering=False, debug=False,
                   num_devices=NC, num_swdge_queues=(1 if str(__import__('os').environ.get('NQ1')) == '1' else 4))
    xtt = (nc.dram_tensor("xt0", [xt_rows_max, R1], BF16, kind="ExternalInput").ap(),
           nc.dram_tensor("xt1", [xt_rows_max, R1], BF16, kind="ExternalInput").ap())
    pool_idx = nc.dram_tensor("pool_idx", [TPC * 128, NIDX // 16], I16,
                              kind="ExternalInput").ap()
    cidx = {}
    for nm in ("convA1", "convB1"):
        cidx[nm] = nc.dram_tensor(nm, [TPC * 128, 1024 // 16], I16,
                                  kind="ExternalInput").ap()
    for nm in ("convA2", "convB2"):
        cidx[nm] = nc.dram_tensor(nm, [TPC * 128, NIDX // 16], I16,
                                  kind="ExternalInput").ap()
    w1kg0 = nc.dram_tensor("w1kg0", [128, C2], BF16, kind="ExternalInput").ap()
    w1kg1 = nc.dram_tensor("w1kg1", [128, C2], BF16, kind="ExternalInput").ap()
    w2k = nc.dram_tensor("w2k", [K * 128, 128], BF16, kind="ExternalInput").ap()
    gvec = nc.dram_tensor("gvec", [C2, 4], F32, kind="ExternalInput").ap()
    out = nc.dram_tensor("out", [B, C2, Vs], F32, kind="ExternalOutput").ap()

    with tile.TileContext(nc) as tc:
        build_body(tc, cfg, xtt, pool_idx, cidx, w1kg0, w1kg1, w2k, gvec, out)
    nc.compile()
    return nc


def build_body(tc, cfg, xt, pool_idx, cidx, w1kg0, w1kg1, w2k, gvec, out):
    nc = tc.nc
    B, C1, C2, K = cfg.B, cfg.C1, cfg.C2, cfg.K
    R1, R2, Vs, TPC, H, NC = cfg.R1, cfg.R2, cfg.Vs, cfg.TPC, cfg.H, cfg.n_cores
    N_stat = float(B * cfg.VC)
    NIDX = K * P
    import os
    NQ = 1 if str(os.environ.get('NQ1')) == '1' else 4
    qn = [0]

    def q():
        qn[0] = (qn[0] + 1) % NQ
        return qn[0]

    ctx = contextlib.ExitStack()
    with ctx:
        dram = ctx.enter_context(tc.tile_pool(name="dram", bufs=1, space="DRAM"))
        sb1 = ctx.enter_context(tc.tile_pool(name="sb1", bufs=1))
        sbg = ctx.enter_context(tc.tile_pool(name="sbg", bufs=2))
        psp = ctx.enter_context(tc.tile_pool(name="psp", bufs=2, space="PSUM"))

        xp_shard = dram.tile([Vs + 1, R1], BF16, name="xp_shard")
        xp_full = dram.tile([(Vs + 1) * NC, R1], BF16, name="xp_full",
                            addr_space="Shared")
        h_raw = dram.tile([Vs, R2], BF16, name="h_raw")
        h_shard = dram.tile([Vs + 1, R2], BF16, name="h_shard")
        h_full = dram.tile([(Vs + 1) * NC, R2], BF16, name="h_full",
                           addr_space="Shared")
        ar_in = dram.tile([C2, 2], F32, name="ar_in")
        ar_out = dram.tile([C2, 2], F32, name="ar_out", addr_space="Shared")
        ar_in2 = dram.tile([C2, 2], F32, name="ar_in2")
        ar_out2 = dram.tile([C2, 2], F32, name="ar_out2", addr_space="Shared")

        w1a = sb1.tile([128, C2], BF16)
        nc.sync.dma_start(w1a[:], w1kg0[:])
        w1b = sb1.tile([128, C2], BF16)
        nc.sync.dma_start(w1b[:], w1kg1[:])
        w2_sb = sb1.tile([128, K * 128], BF16)
        nc.sync.dma_start(
            w2_sb[:].rearrange("p (k e) -> p k e", e=128),
            w2k[:].rearrange("(k p) e -> p k e", p=128))
        g_sb = sb1.tile([C2, 4], F32)
        nc.sync.dma_start(g_sb[:], gvec[:])
        ident = sb1.tile([128, 128], F32)
        make_identity(nc, ident[:])
        zero_bf = sb1.tile([128, R2], BF16)
        nc.vector.memset(zero_bf[:], 0.0)
        alpha_sb = sb1.tile([128, 1], F32)
        nc.vector.memset(alpha_sb[:], ALPHA)
        stats1 = sb1.tile([C2, TPC * 8 * 2], F32)
        stats2 = sb1.tile([128, TPC * 4 * 2], F32)

        # =========== P1: pool ===========
        TH0 = (TPC + 1) // 2
        for t in range(0 if cfg.stop_after == "c1g_nopool" else TPC):
            idx_t = sbg.tile([P, NIDX // 16], I16, tag="pidx", bufs=3)
            nc.sync.dma_start(idx_t[:], pool_idx[t * P:(t + 1) * P, :])
            gp = sbg.tile([P, K * R1], BF16, tag="g1")
            nc.gpsimd.dma_gather(
                out_ap=gp[:].rearrange("p (n e) -> p n e", e=R1),
                in_ap=xt[0][:] if t < TH0 else xt[1][:], idxs_ap=idx_t[:],
                num_idxs=NIDX, num_idxs_reg=NIDX, elem_size=R1, queue_num=q())
            acc = sbg.tile([P, R1], BF16, tag="poolacc", bufs=3)
            with nc.allow_low_precision(reason="bf16 pool sum of 7"):
                nc.vector.tensor_reduce(
                    out=acc[:],
                    in_=bass.AP(gp.tensor, gp[:].offset,
                                [list(gp[:].ap[0]), [1, R1], [R1, K]]),
                    axis=mybir.AxisListType.X, op=OP.add)
            nc.sync.dma_start(xp_shard[t * P:(t + 1) * P, :], acc[:])
        if cfg.stop_after != "c1g_nopool":
            nc.sync.dma_start(xp_shard[Vs:Vs + 1, :], zero_bf[0:1, 0:R1])
        if cfg.stop_after == "pool":
            fin = sbg.tile([P, R1], BF16, tag="fin")
            nc.sync.dma_start(fin[:], xp_shard[0:P, :])
            fin2 = sbg.tile([P, R1], F32, tag="fin2")
            nc.vector.tensor_copy(fin2[:], fin[:])
            nc.sync.dma_start(out[0, 0:64, 0:R1], fin2[0:64, :])
            return
        if cfg.stop_after != "c1g_nopool":
            nc.gpsimd.collective_compute(
                "AllGather", OP.bypass, replica_groups=[list(range(NC))],
                ins=[xp_shard[:].opt()], outs=[xp_full[:].opt()])
        if cfg.stop_after == "ag1":
            fin = sbg.tile([P, R1], BF16, tag="fin")
            nc.sync.dma_start(fin[:], xp_full[0:P, :])
            fin2 = sbg.tile([P, R1], F32, tag="fin2")
            nc.vector.tensor_copy(fin2[:], fin[:])
            nc.sync.dma_start(out[0, 0:64, 0:R1], fin2[0:64, :])
            return

        # =========== C1: conv1 (DVE path, s-slot streams) ===========
        for t in range(TPC):
            iA = sbg.tile([P, 1024 // 16], I16, tag="ia1", bufs=3)
            nc.sync.dma_start(iA[:], cidx["convA1"][t * P:(t + 1) * P, :])
            iB = sbg.tile([P, 1024 // 16], I16, tag="ib1", bufs=3)
            nc.sync.dma_start(iB[:], cidx["convB1"][t * P:(t + 1) * P, :])
            gA = sbg.tile([P, 8 * R1], BF16, tag="gA1")
            nc.gpsimd.dma_gather(
                out_ap=gA[:].rearrange("p (n e) -> p n e", e=R1),
                in_ap=(xp_shard[0:Vs, :] if cfg.stop_after == 'c1g_shard' else xp_full[0:H, :]), idxs_ap=iA[:],
                num_idxs=1024, num_idxs_reg=1024, elem_size=R1, queue_num=q())
            gB = sbg.tile([P, 8 * R1], BF16, tag="gB1")
            nc.gpsimd.dma_gather(
                out_ap=gB[:].rearrange("p (n e) -> p n e", e=R1),
                in_ap=(xp_shard[0:Vs, :] if cfg.stop_after == 'c1g_shard' else (xp_full[0:H, :] if cfg.stop_after == 'c1g_noslice' else xp_full[H:(Vs + 1) * NC, :])), idxs_ap=iB[:],
                num_idxs=1024, num_idxs_reg=1024, elem_size=R1, queue_num=q())
            nc.vector.tensor_tensor(out=gA[:], in0=gA[:], in1=gB[:], op=OP.add)
            if cfg.stop_after in ("c1g", "c1g_noslice", "c1g_shard", "c1g_nopool"):
                continue
            # group-preserving transpose: t1[g*32+c, (s, b, vlo)]
            t1 = sbg.tile([P, 8 * R1], BF16, tag="t1")
            nc.vector.transpose(
                out=t1[:].rearrange("p (s b v) -> p s b v", s=8, b=B),
                in_=gA[:].rearrange("p (s b c) -> p s b c", s=8, b=B))
            psums = [psp.tile([C2, 512], F32, space="PSUM",
                              name=f"ps1_{t}_{vhi}", tag=f"psA_{vhi}")
                     for vhi in range(4)]
            for vhi in range(4):
                for khi in range(2):
                    s = khi * 4 + vhi
                    nc.tensor.matmul(
                        out=psums[vhi][:, :],
                        lhsT=w1a[:, :] if khi == 0 else w1b[:, :],
                        rhs=bass.AP(t1.tensor, t1[:].offset + s * 512,
                                    [list(t1[:].ap[0]), [32, B], [1, 32]]),
                        start=(khi == 0), stop=(khi == 1))
            if cfg.stop_after == "c1mm":
                continue
            for vhi in range(4):
                sc = t * 8 + vhi * 2
                dump = sbg.tile([C2, 512], BF16, tag="dumpb", bufs=3)
                nc.scalar.activation(
                    out=dump[:], in_=psums[vhi][:], func=AF.Identity,
                    accum_out=stats1[:, sc:sc + 1])
                dump2 = sbg.tile([C2, 512], F32, tag="dump", bufs=2)
                nc.scalar.activation(
                    out=dump2[:], in_=psums[vhi][:], func=AF.Square,
                    accum_out=stats1[:, sc + 1:sc + 2])
                ht = sbg.tile([C2, 512], BF16, tag="ht", bufs=4)
                nc.vector.transpose(
                    out=ht[:].rearrange("p (b v) -> p b v", b=B),
                    in_=dump[:].rearrange("p (b v) -> p b v", b=B))
                # ht[(og, vlo), (b, olow)] -> h_raw[v, b*64+og*32+olow]
                for og in range(2):
                    dst = bass.AP(
                        h_raw.tensor,
                        h_raw[:].offset + (t * P + vhi * 32) * R2 + og * 32,
                        [[R2, 32], [64, B], [1, 32]])
                    nc.sync.dma_start(dst, ht[og * 32:(og + 1) * 32, :])

        if cfg.stop_after in ("conv1", "c1g", "c1g_noslice", "c1g_shard", "c1g_nopool", "c1mm"):
            fin = sbg.tile([P, R2], BF16, tag="finb")
            nc.sync.dma_start(fin[:], h_raw[0:P, :])
            fin2 = sbg.tile([P, R2], F32, tag="finb2")
            nc.vector.tensor_copy(fin2[:], fin[:])
            nc.sync.dma_start(out[0, 0:64, 0:R2], fin2[0:64, :])
            return
        # ---- BN1 stats AR + apply + AG ----
        _stats_reduce(nc, sb1, stats1, ar_in, ar_out, NC, "st1")
        glob1 = sb1.tile([C2, 2], F32)
        nc.sync.dma_start(glob1[:], ar_out[:])
        a1, c1 = _bn_coeffs(nc, sb1, glob1, g_sb[:, 0:1], g_sb[:, 1:2],
                            N_stat, "bn1")
        a1m, c1m = _bn_rowmaps(nc, sb1, psp, ident, a1, c1, C2, B, "b1m")
        for t in range(TPC):
            ha = sbg.tile([P, R2], BF16, tag="happly", bufs=3)
            nc.sync.dma_start(ha[:], h_raw[t * P:(t + 1) * P, :])
            nc.vector.tensor_tensor(out=ha[:], in0=ha[:], in1=a1m[:, :],
                                    op=OP.mult)
            nc.vector.tensor_tensor(out=ha[:], in0=ha[:], in1=c1m[:, :],
                                    op=OP.add)
            nc.scalar.activation(out=ha[:], in_=ha[:], func=AF.Prelu,
                                 alpha=alpha_sb[:, :])
            nc.sync.dma_start(h_shard[t * P:(t + 1) * P, :], ha[:])
        nc.sync.dma_start(h_shard[Vs:Vs + 1, :], zero_bf[0:1, :])
        if cfg.stop_after == "bn1":
            fin = sbg.tile([P, R2], BF16, tag="finb")
            nc.sync.dma_start(fin[:], h_shard[0:P, :])
            fin2 = sbg.tile([P, R2], F32, tag="finb2")
            nc.vector.tensor_copy(fin2[:], fin[:])
            nc.sync.dma_start(out[0, 0:64, 0:R2], fin2[0:64, :])
            return
        nc.gpsimd.collective_compute(
            "AllGather", OP.bypass, replica_groups=[list(range(NC))],
            ins=[h_shard[:].opt()], outs=[h_full[:].opt()])

        # =========== C2: conv2 (transpose-gather path) ===========
        for t in range(TPC):
            iA = sbg.tile([P, NIDX // 16], I16, tag="ia2", bufs=3)
            nc.sync.dma_start(iA[:], cidx["convA2"][t * P:(t + 1) * P, :])
            iB = sbg.tile([P, NIDX // 16], I16, tag="ib2", bufs=3)
            nc.sync.dma_start(iB[:], cidx["convB2"][t * P:(t + 1) * P, :])
            gTA = sbg.tile([P, 8 * NIDX], BF16, tag="gTA")
            nc.gpsimd.dma_gather(
                out_ap=gTA[:].rearrange("p (j n) -> p j n", j=8),
                in_ap=h_full[0:H, :], idxs_ap=iA[:],
                num_idxs=NIDX, num_idxs_reg=NIDX, elem_size=R2,
                transpose=True, queue_num=q())
            gTB = sbg.tile([P, 8 * NIDX], BF16, tag="gTB")
            nc.gpsimd.dma_gather(
                out_ap=gTB[:].rearrange("p (j n) -> p j n", j=8),
                in_ap=h_full[H:(Vs + 1) * NC, :], idxs_ap=iB[:],
                num_idxs=NIDX, num_idxs_reg=NIDX, elem_size=R2,
                transpose=True, queue_num=q())
            psums = [psp.tile([128, 512], F32, space="PSUM",
                              name=f"ps2_{t}_{vh}", tag=f"psA_{vh}")
                     for vh in range(2)]
            for vh in range(2):
                for half, gT in ((0, gTA), (1, gTB)):
                    for k in range(K):
                        # rhs: free (8 b_hi, 64 v): cols j*NIDX + k*128 + vh*64 + v
                        rhs = bass.AP(gT.tensor,
                                      gT[:].offset + k * 128 + vh * 64,
                                      [list(gT[:].ap[0]), [NIDX, 8], [1, 64]])
                        nc.tensor.matmul(
                            out=psums[vh][:, :], lhsT=w2_sb[:, k * 128:(k + 1) * 128],
                            rhs=rhs,
                            start=(half == 0 and k == 0),
                            stop=(half == 1 and k == K - 1))
            for vh in range(2):
                sc = t * 4 + vh * 2
                stg = sbg.tile([128, 512], F32, tag="stg", bufs=3)
                nc.scalar.activation(
                    out=stg[:], in_=psums[vh][:], func=AF.Identity,
                    accum_out=stats2[:, sc:sc + 1])
                dump2 = sbg.tile([128, 512], F32, tag="dump", bufs=2)
                nc.scalar.activation(
                    out=dump2[:], in_=psums[vh][:], func=AF.Square,
                    accum_out=stats2[:, sc + 1:sc + 2])
                for bl in range(2):
                    for bh in range(8):
                        b = bh * 2 + bl
                        nc.sync.dma_start(
                            out[b, :, t * P + vh * 64: t * P + (vh + 1) * 64],
                            stg[bl * 64:(bl + 1) * 64, bh * 64:(bh + 1) * 64])

        if cfg.stop_after == "conv2":
            return
        # ---- BN2 stats + fixup ----
        _stats_reduce2(nc, sb1, stats2, ar_in2, ar_out2, NC, "st2")
        glob2 = sb1.tile([C2, 2], F32)
        nc.sync.dma_start(glob2[:], ar_out2[:])
        a2, c2 = _bn_coeffs(nc, sb1, glob2, g_sb[:, 2:3], g_sb[:, 3:4],
                            N_stat, "bn2")
        a2r = sb1.tile([128, 1], F32, name="a2r")
        c2r = sb1.tile([128, 1], F32, name="c2r")
        nc.vector.tensor_copy(a2r[0:C2, :], a2[:])
        nc.vector.tensor_copy(a2r[C2:2 * C2, :], a2[:])
        nc.vector.tensor_copy(c2r[0:C2, :], c2[:])
        nc.vector.tensor_copy(c2r[C2:2 * C2, :], c2[:])
        CH2 = 1024
        for b in range(0, B, 2):
            for v0 in range(0, Vs, CH2):
                vw = min(CH2, Vs - v0)
                fx = sbg.tile([128, CH2], F32, tag="fix", bufs=3)
                nc.sync.dma_start(fx[0:C2, :vw], out[b, :, v0:v0 + vw])
                nc.sync.dma_start(fx[C2:2 * C2, :vw], out[b + 1, :, v0:v0 + vw])
                nc.scalar.activation(out=fx[:, :vw], in_=fx[:, :vw],
                                     func=AF.Prelu, bias=c2r[:], scale=a2r[:],
                                     alpha=alpha_sb[:, :])
                nc.sync.dma_start(out[b, :, v0:v0 + vw], fx[0:C2, :vw])
                nc.sync.dma_start(out[b + 1, :, v0:v0 + vw], fx[C2:2 * C2, :vw])


def _stats_reduce(nc, sb1, stats, ar_in, ar_out, NC, nm):
    C2 = stats.shape[0]
    red = sb1.tile([C2, 2], F32, name=f"{nm}_red")
    sv = stats[:].rearrange("p (s two) -> p two s", two=2)
    nc.vector.tensor_reduce(out=red[:, 0:1], in_=sv[:, 0:1, :],
                            axis=mybir.AxisListType.X, op=OP.add)
    nc.vector.tensor_reduce(out=red[:, 1:2], in_=sv[:, 1:2, :],
                            axis=mybir.AxisListType.X, op=OP.add)
    nc.gpsimd.dma_start(ar_in[:], red[:])
    nc.gpsimd.collective_compute(
        "AllReduce", OP.add, replica_groups=[list(range(NC))],
        ins=[ar_in[:].opt()], outs=[ar_out[:].opt()])


def _stats_reduce2(nc, sb1, stats2, ar_in, ar_out, NC, nm):
    """stats2 is (128=(2bl,64o), S*2): reduce free, then fold b_lo halves."""
    red = sb1.tile([128, 2], F32, name=f"{nm}_redf")
    sv = stats2[:].rearrange("p (s two) -> p two s", two=2)
    nc.vector.tensor_reduce(out=red[:, 0:1], in_=sv[:, 0:1, :],
                            axis=mybir.AxisListType.X, op=OP.add)
    nc.vector.tensor_reduce(out=red[:, 1:2], in_=sv[:, 1:2, :],
                            axis=mybir.AxisListType.X, op=OP.add)
    hi = sb1.tile([64, 2], F32, name=f"{nm}_hi")
    nc.vector.tensor_copy(hi[:], red[64:128, :])
    red64 = sb1.tile([64, 2], F32, name=f"{nm}_red")
    nc.vector.tensor_tensor(out=red64[:], in0=red[0:64, :], in1=hi[:],
                            op=OP.add)
    nc.gpsimd.dma_start(ar_in[:], red64[:])
    nc.gpsimd.collective_compute(
        "AllReduce", OP.add, replica_groups=[list(range(NC))],
        ins=[ar_in[:].opt()], outs=[ar_out[:].opt()])


def _bn_coeffs(nc, sb1, glob, gamma, beta, N, nm):
    C2 = glob.shape[0]
    tmp = sb1.tile([C2, 4], F32, name=f"{nm}_tmp")
    nc.scalar.activation(out=tmp[:, 0:1], in_=glob[:, 0:1], func=AF.Copy,
                         scale=1.0 / N)
    nc.scalar.activation(out=tmp[:, 1:2], in_=glob[:, 1:2], func=AF.Copy,
                         scale=1.0 / N)
    msq = sb1.tile([C2, 1], F32, name=f"{nm}_msq")
    nc.vector.tensor_tensor(out=msq[:], in0=tmp[:, 0:1], in1=tmp[:, 0:1],
                            op=OP.mult)
    nc.vector.tensor_tensor(out=tmp[:, 2:3], in0=tmp[:, 1:2], in1=msq[:],
                            op=OP.subtract)
    std = sb1.tile([C2, 1], F32, name=f"{nm}_std")
    epsb = sb1.tile([C2, 1], F32, name=f"{nm}_eps")
    nc.vector.memset(epsb[:], EPS)
    nc.scalar.activation(out=std[:], in_=tmp[:, 2:3], func=AF.Sqrt, bias=epsb[:])
    rstd = sb1.tile([C2, 1], F32, name=f"{nm}_rstd")
    nc.vector.reciprocal(rstd[:], std[:])
    a = sb1.tile([C2, 1], F32, name=f"{nm}_a")
    nc.vector.tensor_tensor(out=a[:], in0=gamma, in1=rstd[:], op=OP.mult)
    c = sb1.tile([C2, 1], F32, name=f"{nm}_c")
    am = sb1.tile([C2, 1], F32, name=f"{nm}_am")
    nc.vector.tensor_tensor(out=am[:], in0=a[:], in1=tmp[:, 0:1], op=OP.mult)
    nc.vector.tensor_tensor(out=c[:], in0=beta, in1=am[:], op=OP.subtract)
    return a, c


def bcast(ap_obj, pos, count):
    new = bass.AP(ap_obj.tensor, ap_obj.offset, [list(d) for d in ap_obj.ap])
    new.ap.insert(pos + 1, [0, count])
    return new


def _bn_rowmaps(nc, sb1, psp, ident, a, c, C2, B, nm):
    """(128, B*C2) bf16 maps m[p, (b,o)] = vec[o]."""
    aT_ps = psp.tile([1, C2], F32, space="PSUM", name=f"{nm}_aT", tag="psA_0")
    nc.tensor.transpose(out=aT_ps[:], in_=a[:], identity=ident[0:C2, 0:C2])
    aT = sb1.tile([1, C2], F32, name=f"{nm}_aTs")
    nc.vector.tensor_copy(aT[:], aT_ps[:])
    cT_ps = psp.tile([1, C2], F32, space="PSUM", name=f"{nm}_cT", tag="psA_1")
    nc.tensor.transpose(out=cT_ps[:], in_=c[:], identity=ident[0:C2, 0:C2])
    cT = sb1.tile([1, C2], F32, name=f"{nm}_cTs")
    nc.vector.tensor_copy(cT[:], cT_ps[:])
    R2 = B * C2
    am1 = sb1.tile([1, R2], BF16, name=f"{nm}_amap1")
    cm1 = sb1.tile([1, R2], BF16, name=f"{nm}_cmap1")
    nc.vector.tensor_copy(am1[0:1, :].rearrange("p (b e) -> p b e", e=C2),
                          bcast(aT[0:1, :], 0, B))
    nc.vector.tensor_copy(cm1[0:1, :].rearrange("p (b e) -> p b e", e=C2),
                          bcast(cT[0:1, :], 0, B))
    am = sb1.tile([128, R2], BF16, name=f"{nm}_amap")
    cm = sb1.tile([128, R2], BF16, name=f"{nm}_cmap")
    nc.gpsimd.partition_broadcast(am[:], am1[:])
    nc.gpsimd.partition_broadcast(cm[:], cm1[:])
    return am, cm


# ============================================================================
import concourse.bass_utils as bass_utils

_CACHE = {}


def kernel(x, w1, b1, g1, beta1, w2, b2, g2, beta2,
           conv_neigh_indices, down_neigh_indices):
    """DownBlock: IcoPool(mean) -> (conv-BN-LReLU) x2 on 8 trn2 NeuronCores."""
    x = np.asarray(x, np.float32)
    B, C1, VF = x.shape
    VC, K = np.asarray(conv_neigh_indices).shape
    C2 = np.asarray(w1).shape[0]
    cfg = Cfg(B=B, C1=C1, C2=C2, K=K, VF=VF, VC=VC, n_cores=8)
    xt_rows, per_core = host_prep(
        cfg, x, w1, w2, g1, beta1, g2, beta2,
        conv_neigh_indices, down_neigh_indices)
    key = (B, C1, C2, K, VF, VC, xt_rows)
    if key not in _CACHE:
        _CACHE[key] = build(cfg, xt_rows)
    nc = _CACHE[key]
    res = bass_utils.run_bass_kernel_spmd(
        nc, per_core, core_ids=list(range(cfg.n_cores)))
    out = np.concatenate([r["out"] for r in res.results], axis=2)[:, :, :VC]
    return np.ascontiguousarray(out, dtype=np.float32)
